# revision 1
# baseline (speedup 1.0000x reference)
"""Trainium2 Bass kernel for a dense transformer attention layer.

Computes, for x:[2,2048,1024] (B=2, T=2048, D=1024, H=16 heads, dk=64,
dff=4096):
    q,k,v = split_heads(x@Wq+bq), ...     (per-head dims 64)
    attn  = softmax(causal_mask(q k^T / 8)) v
    x1    = LN(x + 2*attn; g1, be1)
    out   = LN(x1 + 2*(relu(x1@W1+b1)@W2+b2); g2, be2)

Distribution over 8 NeuronCores:
  - QKV + attention: head-parallel (2 heads per core), all tokens.
    Identical causal loop structure on every core (SPMD-safe).
  - One AllToAll reshards attention output from head-major to
    token-major (each core keeps a 512-token slice, all 1024 channels).
  - LN1 + FFN + LN2: token-parallel (512 tokens per core, full dff).
  - No other collectives.

On-chip layout is feature-major ([channels, tokens]) everywhere, so no
activation transposes are needed for matmuls; attention scores are
computed transposed ([k, q]) so the softmax denominator falls out of the
PV matmul via an extra ones-column in the token-major V tiles.
"""
import os
import math
from contextlib import ExitStack

import numpy as np

import concourse.bass as bass
import concourse.tile as tile
from concourse import bacc, mybir
from concourse.bass_utils import run_bass_kernel_spmd

F32 = mybir.dt.float32
F32R = mybir.dt.float32r
BF16 = mybir.dt.bfloat16
BF = np.dtype("bfloat16")

NCORES = 8
B, T, C, DK, H, DFF = 2, 2048, 1024, 64, 16, 4096
TOK = B * T            # 4096 tokens
TSL = TOK // NCORES    # 512 tokens per core (post-attention shard)
CT = C // 128          # 8 channel tiles
EPS = 1e-5

_CACHE = {}
LAST_EXEC_NS = None


def _emit(nc, causal: bool, masked: bool):
    """Emit the SPMD program. causal: skip/selective-mask causal blocks.
    masked: add a generic additive mask input (maskT, [k,q] layout)."""
    dt_in = nc.dram_tensor
    xT = dt_in("xT", [CT, 128, TOK], BF16, kind="ExternalInput").ap()
    xres = dt_in("xres", [CT, 128, TSL], F32, kind="ExternalInput").ap()
    wq = dt_in("wq", [CT, 128, 128], BF16, kind="ExternalInput").ap()
    wk = dt_in("wk", [CT, 128, 128], BF16, kind="ExternalInput").ap()
    wv = dt_in("wv", [CT, 128, 128], BF16, kind="ExternalInput").ap()
    bqkv = dt_in("bqkv", [128, 3], F32, kind="ExternalInput").ap()
    w1 = dt_in("w1", [C, DFF], BF16, kind="ExternalInput").ap()
    b1 = dt_in("b1", [128, DFF // 128], F32, kind="ExternalInput").ap()
    w2 = dt_in("w2", [DFF, C], BF16, kind="ExternalInput").ap()
    b2x2 = dt_in("b2x2", [128, CT], F32, kind="ExternalInput").ap()
    g1v = dt_in("g1v", [128, CT], F32, kind="ExternalInput").ap()
    be1v = dt_in("be1v", [128, CT], F32, kind="ExternalInput").ap()
    g2v = dt_in("g2v", [128, CT], F32, kind="ExternalInput").ap()
    be2v = dt_in("be2v", [128, CT], F32, kind="ExternalInput").ap()
    amask = None
    if masked:
        # additive mask, transposed: amask[kt][k 128, q 2048] (bf16, 0/-30000)
        amask = dt_in("amask", [T // 128, 128, T], BF16,
                      kind="ExternalInput").ap()
    out = dt_in("out", [CT, 128, TSL], F32, kind="ExternalOutput").ap()

    NQC = T // 512  # 4 q-chunks of 512 per batch

    with tile.TileContext(nc) as tc, ExitStack() as ctx:
        persist = ctx.enter_context(tc.tile_pool(name="persist", bufs=1))
        dram = ctx.enter_context(tc.tile_pool(name="dram", bufs=1,
                                              space="DRAM"))

        # ---- persistent SBUF tensors (small; live whole kernel) ----
        ident = persist.tile([128, 128], BF16, name="ident", tag="ident")
        ones128 = persist.tile([128, 128], F32R, name="ones128", tag="ones128")
        onesf = persist.tile([128, 128], F32, name="onesf", tag="onesf")
        bias3 = persist.tile([128, 3], F32, name="bias3", tag="bias3")

        nc.sync.dma_start(bias3[:], bqkv[:])
        nc.vector.memset(onesf[:], 1.0)
        nc.vector.tensor_copy(ones128[:], onesf[:])
        nc.gpsimd.memset(ident[:], 0.0)
        nc.gpsimd.affine_select(
            out=ident[:], in_=ident[:], compare_op=mybir.AluOpType.not_equal,
            fill=1.0, base=0, pattern=[[-1, 128]], channel_multiplier=1)

        a2a_in = dram.tile([NCORES, 128, TSL], BF16)
        a2a_out = dram.tile([NCORES, 128, TSL], BF16)

        # W1 first half preloaded at kernel start so FFN1 never waits on DMA
        wearly = ctx.enter_context(tc.tile_pool(name="wearly", bufs=1))
        w1pre = {}
        for mg in range(4):
            for kt in range(CT):
                w1pre[(mg, kt)] = wearly.tile(
                    [128, 512], BF16, name=f"w1p_{mg}_{kt}",
                    tag=f"w1p_{mg}_{kt}")

        # ---- phases 1+2: QKV projections + attention ----
        with ExitStack() as c12:
            cattn = c12.enter_context(tc.tile_pool(name="cattn", bufs=1))
            # zero-padded per-head q (full-K scores matmuls -> FWL fast)
            qp = [cattn.tile([128, TOK], BF16, name=f"qp{h}", tag=f"qp{h}")
                  for h in range(2)]
            kT = cattn.tile([128, TOK], BF16, name="kT", tag="kT")
            # token-major v tiles; cols h*65..h*65+64 = [v_h | ones],
            # cols 130..199 zero padding so lhsT can always be 128 wide
            vtok = [cattn.tile([128, 200], BF16, name=f"vtok{i}",
                               tag=f"vtok{i}") for i in range(TOK // 128)]

            with ExitStack() as c1:
                xpool = c1.enter_context(tc.tile_pool(name="xpool", bufs=1))
                ps = c1.enter_context(tc.tile_pool(name="ps_qkv", bufs=4,
                                                   space="PSUM"))
                pst = c1.enter_context(tc.tile_pool(name="ps_tr", bufs=4,
                                                    space="PSUM"))
                wpool = c1.enter_context(tc.tile_pool(name="wqkv", bufs=1))
                vTf = c1.enter_context(tc.tile_pool(name="vTf", bufs=1))
                vT = vTf.tile([128, TOK], BF16, name="vT", tag="vT")

                # weights first: small DMAs so the first matmul starts early
                wts = []
                for wi, wdram in enumerate((wq, wk, wv)):
                    wt = [wpool.tile([128, 128], BF16, name=f"w{wi}_{kt}",
                                     tag=f"w{wi}_{kt}") for kt in range(CT)]
                    for kt in range(CT):
                        nc.sync.dma_start(wt[kt][:], wdram[kt])
                    wts.append(wt)
                xt = [xpool.tile([128, TOK], BF16, name=f"xt{i}",
                                 tag=f"xt{i}") for i in range(CT)]
                for i in range(CT):
                    nc.sync.dma_start(xt[i][:], xT[i])
                for (mg, kt), tpre in w1pre.items():
                    nc.scalar.dma_start(
                        tpre[:], w1[kt * 128:(kt + 1) * 128,
                                    mg * 512:(mg + 1) * 512])

                nc.vector.memset(qp[0][64:128, :], 0.0)
                nc.vector.memset(qp[1][0:64, :], 0.0)

                for wi, (wdram, brow) in enumerate(((wq, 0), (wk, 1),
                                                    (wv, 2))):
                    wt = wts[wi]
                    for ch in range(TOK // 512):
                        cs = slice(ch * 512, (ch + 1) * 512)
                        p = ps.tile([128, 512], F32)
                        for kt in range(CT):
                            nc.tensor.matmul(
                                p[:], wt[kt][:], xt[kt][:, cs],
                                start=(kt == 0), stop=(kt == CT - 1))
                        if wi == 0:  # q: split heads into padded tiles
                            nc.vector.tensor_scalar_add(
                                qp[0][0:64, cs], p[0:64, :],
                                bias3[0:64, 0:1])
                            nc.vector.tensor_scalar_add(
                                qp[1][64:128, cs], p[64:128, :],
                                bias3[64:128, 0:1])
                        else:
                            dst = kT if wi == 1 else vT
                            nc.vector.tensor_scalar_add(
                                dst[:, cs], p[:],
                                bias3[:, brow:brow + 1])

                # transpose v to token-major tiles, insert ones columns
                for j in range(TOK // 128):
                    pt = pst.tile([128, 128], BF16)
                    nc.tensor.matmul(pt[:], vT[:, j * 128:(j + 1) * 128],
                                     ident[:], is_transpose=True,
                                     start=True, stop=True)
                    nc.vector.memset(vtok[j][:, 130:200], 0.0)
                    for lh in range(2):
                        nc.vector.tensor_copy(
                            vtok[j][:, lh * 65:lh * 65 + 64],
                            pt[:, lh * 64:(lh + 1) * 64])
                    nc.vector.memset(vtok[j][:, 64:65], 1.0)
                    nc.vector.memset(vtok[j][:, 129:130], 1.0)

            # ---- phase 2: attention (scores transposed [k, q]) ----
            with ExitStack() as c2:
                ps_sc = c2.enter_context(tc.tile_pool(name="ps_sc", bufs=3,
                                                      space="PSUM"))
                ps_pv = c2.enter_context(tc.tile_pool(name="ps_pv", bufs=3,
                                                      space="PSUM"))
                bcp = c2.enter_context(tc.tile_pool(name="bcp", bufs=4))
                ptp = c2.enter_context(tc.tile_pool(name="ptp", bufs=4))
                stp = c2.enter_context(tc.tile_pool(name="stage", bufs=1))
                mkp = c2.enter_context(tc.tile_pool(name="maskp", bufs=3))

                ps_bc = c2.enter_context(tc.tile_pool(name="ps_bc",
                                                      bufs=2, space="PSUM"))
                stage = [[stp.tile([64, TSL], BF16, name=f"stage{d}_{h}",
                                   tag=f"stage{d}_{h}") for h in range(2)]
                         for d in range(NCORES)]
                denom = stp.tile([16, TSL], F32, name="denom", tag="denom")
                rec16 = stp.tile([16, TSL], F32, name="rec16", tag="rec16")
                ones1 = stp.tile([1, 64], F32, name="ones1", tag="ones1")
                nc.vector.memset(ones1[:], 1.0)

                for b in range(B):
                    for qc in range(NQC):
                        d = b * NQC + qc        # dest core / token chunk
                        q0 = b * T + qc * 512   # global token col of q chunk
                        nkt = (qc + 1) * 4 if causal else T // 128
                        for lh in range(2):
                            pv = ps_pv.tile([128, TSL], F32)
                            for kt in range(nkt):
                                kc = b * T + kt * 128
                                sc = ps_sc.tile([128, 512], F32)
                                nc.tensor.matmul(
                                    sc[:], kT[:, kc:kc + 128],
                                    qp[lh][:, q0:q0 + 512],
                                    start=True, stop=True)
                                if masked:
                                    mkt = mkp.tile([128, 512], BF16)
                                    nc.sync.dma_start(
                                        mkt[:],
                                        amask[kt, :,
                                              qc * 512:(qc + 1) * 512])
                                    nc.vector.tensor_add(sc[:], sc[:],
                                                         mkt[:])
                                pt = ptp.tile([128, 512], BF16)
                                nc.scalar.activation(
                                    pt[:], sc[:],
                                    mybir.ActivationFunctionType.Exp)
                                if causal and kt >= 4 * qc:
                                    # zero entries with k_global > q_global
                                    nc.gpsimd.affine_select(
                                        out=pt[:], in_=pt[:],
                                        compare_op=mybir.AluOpType.is_ge,
                                        fill=0.0, base=qc * 512 - kt * 128,
                                        pattern=[[1, 512]],
                                        channel_multiplier=-1)
                                nc.tensor.matmul(
                                    pv[:], vtok[(b * T) // 128 + kt]
                                    [:, lh * 65:lh * 65 + 128], pt[:],
                                    start=(kt == 0), stop=(kt == nkt - 1))
                            # stash unnormalized attn + denominator row
                            nc.vector.tensor_copy(stage[d][lh][:],
                                                  pv[0:64, :])
                            den1 = bcp.tile([1, TSL], F32,
                                             name="den1", tag="den1")
                            nc.vector.tensor_copy(den1[:], pv[64:65, :])
                            nc.sync.dma_start(
                                denom[2 * d + lh:2 * d + lh + 1, :],
                                den1[:])

                # batched normalization; PE ones-matmuls broadcast 1/denom
                nc.vector.reciprocal(rec16[:], denom[:])
                for d in range(NCORES):
                    for lh in range(2):
                        rec1 = bcp.tile([1, TSL], F32, name="rec1",
                                        tag="rec1")
                        nc.sync.dma_start(
                            rec1[:], rec16[2 * d + lh:2 * d + lh + 1, :])
                        bc = ps_bc.tile([64, TSL], F32, name="bc",
                                        tag="bc")
                        nc.tensor.matmul(bc[:], ones1[:], rec1[:],
                                         start=True, stop=True)
                        nc.vector.tensor_mul(stage[d][lh][:],
                                             stage[d][lh][:], bc[:])
                        nc.sync.dma_start(
                            a2a_in[d, lh * 64:lh * 64 + 64, :],
                            stage[d][lh][:])

                nc.gpsimd.collective_compute(
                    "AllToAll", mybir.AluOpType.bypass,
                    replica_groups=[list(range(NCORES))],
                    ins=[a2a_in.opt()], outs=[a2a_out.opt()])

        # ---- phase 3: residual + LN1 (token slice, feature-major) ----
        x1f = [persist.tile([128, TSL], F32, name=f"x1f{i}", tag=f"x1f{i}")
               for i in range(CT)]
        x1b = [persist.tile([128, TSL], BF16, name=f"x1b{i}", tag=f"x1b{i}")
               for i in range(CT)]

        def layer_norm(zf, g_dram, be_dram, dst_f32, dst_bf16, pools,
                       col=None):
            ps_ln, lnp = pools
            if col is None:
                col = slice(0, TSL)
            W = col.stop - col.start
            zf = [z[:, col] for z in zf]
            dst_f32 = [t[:, col] for t in dst_f32]
            if dst_bf16 is not None:
                dst_bf16 = [t[:, col] for t in dst_bf16]
            TW = W
            epst = lnp.tile([128, 1], F32, name="epst", tag="epst")
            nc.vector.memset(epst[:], EPS)
            sum_ps = ps_ln.tile([128, TW], F32, name="sum_ps", tag="sum_ps")
            for i in range(CT):
                nc.tensor.matmul(sum_ps[:], ones128[:], zf[i][:],
                                 start=(i == 0), stop=(i == CT - 1))
            sq_ps = ps_ln.tile([128, TW], F32, name="sq_ps", tag="sq_ps")
            for i in range(CT):
                zsq = lnp.tile([128, TW], F32R, name="zsq", tag="zsq")
                nc.scalar.square(zsq[:], zf[i][:])
                nc.tensor.matmul(sq_ps[:], ones128[:], zsq[:],
                                 start=(i == 0), stop=(i == CT - 1))
            mu = lnp.tile([128, TW], F32, name="mu", tag="mu")
            nc.vector.tensor_scalar_mul(mu[:], sum_ps[:], 1.0 / C)
            musq = lnp.tile([128, TW], F32, name="musq", tag="musq")
            nc.vector.tensor_mul(musq[:], mu[:], mu[:])
            var = lnp.tile([128, TW], F32, name="var", tag="var")
            nc.vector.scalar_tensor_tensor(
                var[:], sq_ps[:], 1.0 / C, musq[:],
                op0=mybir.AluOpType.mult, op1=mybir.AluOpType.subtract)
            std = lnp.tile([128, TW], F32, name="std", tag="std")
            nc.scalar.activation(std[:], var[:],
                                 mybir.ActivationFunctionType.Sqrt,
                                 bias=epst[:])
            rstd = lnp.tile([128, TW], F32, name="rstd", tag="rstd")
            nc.vector.reciprocal(rstd[:], std[:])
            gt = lnp.tile([128, CT], F32, name="gt", tag="gt")
            bt = lnp.tile([128, CT], F32, name="bt", tag="bt")
            nc.sync.dma_start(gt[:], g_dram[:])
            nc.sync.dma_start(bt[:], be_dram[:])
            for i in range(CT):
                t = lnp.tile([128, TW], F32, name="lnt", tag="lnt")
                nc.vector.tensor_sub(t[:], zf[i][:], mu[:])
                t2 = lnp.tile([128, TW], F32, name="lnt2", tag="lnt2")
                nc.vector.tensor_mul(t2[:], t[:], rstd[:])
                nc.scalar.activation(dst_f32[i][:], t2[:],
                                     mybir.ActivationFunctionType.Identity,
                                     bias=bt[:, i:i + 1], scale=gt[:, i:i + 1])
                if dst_bf16 is not None:
                    nc.vector.tensor_copy(dst_bf16[i][:], dst_f32[i][:])

        with ExitStack() as c3:
            ps_ln = c3.enter_context(tc.tile_pool(name="ps_ln", bufs=2,
                                                  space="PSUM"))
            lnp = c3.enter_context(tc.tile_pool(name="lnp", bufs=2))
            xrp = c3.enter_context(tc.tile_pool(name="xrp", bufs=1))
            z1 = [xrp.tile([128, TSL], F32R, name=f"z1{i}", tag=f"z1{i}")
                  for i in range(CT)]
            for i in range(CT):
                xr = lnp.tile([128, TSL], F32, name="xr", tag="xr")
                nc.sync.dma_start(xr[:], xres[i])
                za = lnp.tile([128, TSL], BF16, name="za", tag="za")
                nc.sync.dma_start(za[:], a2a_out[i])
                nc.vector.scalar_tensor_tensor(
                    z1[i][:], za[:], 2.0, xr[:],
                    op0=mybir.AluOpType.mult, op1=mybir.AluOpType.add)
            for hf in range(2):
                layer_norm(z1, g1v, be1v, x1f, x1b, (ps_ln, lnp),
                           col=slice(hf * (TSL // 2), (hf + 1) * (TSL // 2)))

        # ---- phase 4: FFN (token slice, full dff) ----
        with ExitStack() as c4:
            ps_f = c4.enter_context(tc.tile_pool(name="ps_f", bufs=4,
                                                 space="PSUM"))
            wp = c4.enter_context(tc.tile_pool(name="wp", bufs=3))
            hp = c4.enter_context(tc.tile_pool(name="hp", bufs=1))
            bp = c4.enter_context(tc.tile_pool(name="bp", bufs=1))
            h = [hp.tile([128, TSL], BF16, name=f"h{m}", tag=f"h{m}")
                 for m in range(DFF // 128)]
            b1t = bp.tile([128, DFF // 128], F32, name="b1t", tag="b1t")
            nc.sync.dma_start(b1t[:], b1[:])

            for mg in range(DFF // 512):  # 8 groups of 4 dff tiles
                if mg < 4:
                    wt = [w1pre[(mg, kt)] for kt in range(CT)]
                else:
                    wt = [wp.tile([128, 512], BF16, name=f"w1_{kt}",
                                  tag=f"w1_{kt}", bufs=4)
                          for kt in range(CT)]
                    for kt in range(CT):
                        nc.sync.dma_start(
                            wt[kt][:], w1[kt * 128:(kt + 1) * 128,
                                          mg * 512:(mg + 1) * 512])
                for mi in range(4):
                    m = mg * 4 + mi
                    for hf in range(2):
                        cs = slice(hf * (TSL // 2), (hf + 1) * (TSL // 2))
                        p = ps_f.tile([128, TSL // 2], F32, name="p",
                                      tag="p", bufs=2)
                        for kt in range(CT):
                            nc.tensor.matmul(
                                p[:], wt[kt][:, mi * 128:(mi + 1) * 128],
                                x1b[kt][:, cs],
                                start=(kt == 0), stop=(kt == CT - 1))
                        nc.vector.tensor_scalar(
                            h[m][:, cs], p[:], b1t[:, m:m + 1], 0.0,
                            mybir.AluOpType.add, mybir.AluOpType.max)

            # FFN2 + residual into z2
            z2 = [hp.tile([128, TSL], F32R, name=f"z2{i}", tag=f"z2{i}")
                  for i in range(CT)]
            b2t = bp.tile([128, CT], F32, name="b2t", tag="b2t")
            nc.sync.dma_start(b2t[:], b2x2[:])
            NKT2 = DFF // 128
            for cg in range(C // 512):  # 2 groups of 4 C tiles
                pcs = [ps_f.tile([128, TSL], F32, name=f"pc{ci}",
                                 tag=f"pc{ci}", bufs=1) for ci in range(4)]
                for kt in range(NKT2):
                    w2t = wp.tile([128, 512], BF16, name="w2t", tag="w2t",
                                  bufs=12)
                    nc.sync.dma_start(
                        w2t[:], w2[kt * 128:(kt + 1) * 128,
                                   cg * 512:(cg + 1) * 512])
                    for ci in range(4):
                        nc.tensor.matmul(
                            pcs[ci][:], w2t[:, ci * 128:(ci + 1) * 128],
                            h[kt][:], start=(kt == 0),
                            stop=(kt == NKT2 - 1))
                for ci in range(4):
                    i = cg * 4 + ci
                    t = hp.tile([128, TSL], F32, name="ffo", tag="ffo")
                    nc.scalar.activation(t[:], pcs[ci][:],
                                         mybir.ActivationFunctionType.Identity,
                                         bias=b2t[:, i:i + 1], scale=2.0)
                    nc.vector.tensor_add(z2[i][:], t[:], x1f[i][:])

            # ---- phase 5: LN2 + output ----
            ps_ln2 = c4.enter_context(tc.tile_pool(name="ps_ln2", bufs=1,
                                                   space="PSUM"))
            lnp2 = c4.enter_context(tc.tile_pool(name="lnp2", bufs=2))
            outf = [hp.tile([128, TSL], F32, name=f"of{i}", tag=f"of{i}")
                    for i in range(CT)]
            layer_norm(z2, g2v, be2v, outf, None, (ps_ln2, lnp2))
            for i in range(CT):
                nc.sync.dma_start(out[i], outf[i][:])


def _build(causal: bool, masked: bool):
    key = (causal, masked)
    if key in _CACHE:
        return _CACHE[key]
    nc = bacc.Bacc("TRN2", target_bir_lowering=False, debug=False,
                   num_devices=NCORES)
    _emit(nc, causal, masked)
    nc.compile()
    _CACHE[key] = nc
    return nc


def kernel(x, attention_mask, Wq, bq, Wk, bk, Wv, bv, W1, b1, W2, b2,
           g1, be1, g2, be2):
    global LAST_EXEC_NS
    f32 = np.float32
    x = np.asarray(x, f32).reshape(TOK, C)
    xT = np.ascontiguousarray(x.T)                      # [C, TOK]
    mask = np.asarray(attention_mask).reshape(T, T)

    causal = bool(np.array_equal(
        mask != 0, np.tril(np.ones((T, T), dtype=bool))))
    masked = (not causal) and not bool((mask != 0).all())

    Wq = np.asarray(Wq, f32); Wk = np.asarray(Wk, f32)
    Wv = np.asarray(Wv, f32); W1 = np.asarray(W1, f32)
    W2 = np.asarray(W2, f32)
    bq = np.asarray(bq, f32); bk = np.asarray(bk, f32)
    bv = np.asarray(bv, f32); b1 = np.asarray(b1, f32)
    b2 = np.asarray(b2, f32)
    scale = 1.0 / math.sqrt(DK)

    shared = dict(
        xT=xT.reshape(CT, 128, TOK).astype(BF),
        w1=W1.astype(BF),
        b1=np.ascontiguousarray(b1.reshape(DFF // 128, 128).T),
        w2=W2.astype(BF),
        b2x2=np.ascontiguousarray((2.0 * b2).reshape(CT, 128).T),
        g1v=np.ascontiguousarray(np.asarray(g1, f32).reshape(CT, 128).T),
        be1v=np.ascontiguousarray(np.asarray(be1, f32).reshape(CT, 128).T),
        g2v=np.ascontiguousarray(np.asarray(g2, f32).reshape(CT, 128).T),
        be2v=np.ascontiguousarray(np.asarray(be2, f32).reshape(CT, 128).T),
    )
    if masked:
        add = np.where(mask != 0, 0.0, -30000.0).astype(f32)
        shared["amask"] = np.ascontiguousarray(add.T).reshape(
            T // 128, 128, T).astype(BF)

    in_maps = []
    for c in range(NCORES):
        hs = slice(c * 128, (c + 1) * 128)
        m = dict(shared)
        m["xres"] = np.ascontiguousarray(
            xT[:, c * TSL:(c + 1) * TSL]).reshape(CT, 128, TSL)
        m["wq"] = np.ascontiguousarray(
            Wq[:, hs] * scale).reshape(CT, 128, 128).astype(BF)
        m["wk"] = np.ascontiguousarray(Wk[:, hs]).reshape(
            CT, 128, 128).astype(BF)
        m["wv"] = np.ascontiguousarray(Wv[:, hs]).reshape(
            CT, 128, 128).astype(BF)
        m["bqkv"] = np.ascontiguousarray(
            np.stack([bq[hs] * scale, bk[hs], bv[hs]], axis=1)).astype(f32)
        in_maps.append(m)

    nc = _build(causal, masked)
    trace = os.environ.get("KERNEL_TRACE") == "1"
    if trace:
        try:
            import prof_shim
            prof_shim.install()
            res = run_bass_kernel_spmd(
                nc, in_maps, list(range(NCORES)), trace=True,
                tmpdir=os.environ.get("KERNEL_TRACE_DIR"))
            LAST_EXEC_NS = res.exec_time_ns
        except Exception:
            res = run_bass_kernel_spmd(nc, in_maps, list(range(NCORES)))
    else:
        res = run_bass_kernel_spmd(nc, in_maps, list(range(NCORES)))

    outT = np.concatenate(
        [res.results[c]["out"].reshape(C, TSL) for c in range(NCORES)],
        axis=1)                                          # [C, TOK]
    return np.ascontiguousarray(outT.T).reshape(B, T, C).astype(f32)

